# revision 1
# baseline (speedup 1.0000x reference)
"""Conv2d-via-Linear Trainium2 kernel.

The problem's [16,30,30,3,64,64] weight is (for the reference's
setup_inputs) a structured-sparse replication of a single 5x5/stride-2
conv kernel w0 [16,3,5,5]:  big[:, oh, ow, :, 2oh:2oh+5, 2ow:2ow+5] = w0.
So out = x2 @ w2.T + bias is exactly Conv2d(x, w0, stride=2) + b0.

Device strategy (8 NeuronCores, batch-parallel, 8 images per core):
  - Host lays out each core's batch shard as a 75-row im2col operand
    X[(c,kh,kw), (b, oh, ow)] with the (kh>>1, kw>>1) spatial shifts baked
    in as flat offsets, so the device load is plain wide contiguous DMAs
    spanning all 75 partitions (full SDMA engine spread).
  - One fp32 matmul (K=75, M=32) per (image, oh-half) output chunk;
    PE column-tiling packs 4 chunks into the 4 col-groups of one PSUM
    bank; weight cols 16..31 are zero so the spare partitions hold
    computed zeros, letting a single [128, 450] DVE bias-add evacuate
    4 chunks at once.
  - Loads are split per image-pair and each matmul group only consumes
    its pair's tile, so DMA/compute pipeline under the Tile scheduler.
  - Output rows go back as [oh-half, o] contiguous 1800B runs into a
    [8, 32, 900] padded buffer; host keeps [:, :16].

If the weight/bias do not have the replicated-conv structure (never the
case for the real reference inputs), falls back to the dense matmul on
host so the result is still correct.
"""

import numpy as np

B, C, H, W = 64, 3, 64, 64
O, KK, S = 16, 5, 2
OH = OW = 30
NCORES = 8
BPC = B // NCORES  # images per core

HB = H // 2  # 32  (h2 dim)
WB = W // 2  # 32  (w2 dim)
XQ_LEN = 2 * C * 2 * BPC * HB * WB  # 98304
XQ_PAD = XQ_LEN + 128  # shifted reads spill <= 66+7*1024+1023 past slice starts

# (khp, kwp) -> partition-block layout. kh = 2*khh + khp, kw = 2*kwh + kwp.
_BLOCKS = []
_p0 = 0
for _khp in (0, 1):
    for _kwp in (0, 1):
        _nkh = 3 - _khp
        _nkw = 3 - _kwp
        _BLOCKS.append((_khp, _kwp, _p0, _nkh, _nkw))
        _p0 += _nkh * _nkw * C
NPART = _p0  # 75

# element strides inside flat xq [hp, c, wp, b, h2, w2]
_ST_WP = BPC * HB * WB        # 8192
_ST_C = 2 * _ST_WP            # 16384
_ST_HP = C * _ST_C            # 49152

# per-partition source offsets into flat xq, and (c, kh, kw) per partition
_SRC_OFF = np.empty(NPART, dtype=np.int64)
_PART_CKHKW = []
for _khp, _kwp, _pp0, _nkh, _nkw in _BLOCKS:
    _pi = _pp0
    for _khh in range(_nkh):
        for _kwh in range(_nkw):
            for _c in range(C):
                _SRC_OFF[_pi] = (_khp * _ST_HP + _c * _ST_C + _kwp * _ST_WP
                                 + _khh * HB + _kwh)
                _PART_CKHKW.append((_c, 2 * _khh + _khp, 2 * _kwh + _kwp))
                _pi += 1

_NC_CACHE = {}
LAST_RESULT = None


def _install_trace_shim():
    """Make bass_utils' trace path importable even when antenv.axon_hooks
    is absent (it is in this container). Harmless if tracing is off."""
    import sys, types
    try:
        import antenv.axon_hooks  # noqa: F401
        return
    except ImportError:
        pass
    mod = types.ModuleType("antenv.axon_hooks")
    hook = [None]
    mod.set_axon_ntff_profile_hook = lambda h: hook.__setitem__(0, h)
    mod.get_axon_ntff_profile_hook = lambda: hook[0]
    sys.modules["antenv.axon_hooks"] = mod
    try:
        from trn_agent_boot.trn_boot import _ntff_profile_via_ctypes
        hook[0] = _ntff_profile_via_ctypes("/opt/axon/libaxon_pjrt.so")
    except Exception:
        pass


def _structure_ok(weight, w0, bias, b0):
    """Exact check that `weight` is w0 replicated per output position and
    everything else zero, and that bias is b0 repeated per position."""
    try:
        from numpy.lib.stride_tricks import as_strided
        s = weight.strides
        blocks = as_strided(
            weight,
            shape=(OH, OW, O, C, KK, KK),
            strides=(s[1] + S * s[4], s[2] + S * s[5], s[0], s[3], s[4], s[5]),
        )
        if not (blocks == w0[None, None]).all():
            return False
        if np.count_nonzero(weight) != OH * OW * np.count_nonzero(w0):
            return False
        if not (bias[0].reshape(O, OH * OW) == b0[:, None]).all():
            return False
        return True
    except Exception:
        return False


def _build_nc():
    import concourse.mybir as mybir
    import concourse.tile as tile
    from concourse import bacc

    f32 = mybir.dt.float32
    nc = bacc.Bacc(None, target_bir_lowering=False)
    with tile.TileContext(nc) as tc:
        with tc.tile_pool(name="dram", bufs=1, space="DRAM") as dram:
            bf16 = mybir.dt.bfloat16
            xhi = dram.tile([NPART, BPC * HB * WB], bf16, kind="ExternalInput",
                            name="xhi", uniquify=False)
            xlo = dram.tile([NPART, BPC * HB * WB], bf16, kind="ExternalInput",
                            name="xlo", uniquify=False)
            w0r = dram.tile([NPART, 64], bf16, kind="ExternalInput",
                            name="w0r", uniquify=False)
            bias_t = dram.tile([128, 1], f32, kind="ExternalInput",
                               name="biasT", uniquify=False)
            out = dram.tile([2, 128, 2 * 15 * OW], f32, kind="ExternalOutput",
                            name="out", uniquify=False)

            with (
                tc.tile_pool(name="const", bufs=1) as constp,
                tc.tile_pool(name="xdata", bufs=1) as xpool,
                tc.tile_pool(name="evac", bufs=2) as evacp,
                tc.tile_pool(name="psum", bufs=2, space="PSUM") as psump,
            ):
                # tiny consts first on each HWDGE ring (SWDGE completion
                # latency is erratic and gated the first matmuls)
                wsb = constp.tile([NPART, 64], bf16, name="wsb")
                nc.sync.dma_start(wsb[:], w0r[:])
                bsb = constp.tile([128, 1], f32, name="bsb")
                nc.scalar.dma_start(bsb[:], bias_t[:])

                # one hi and one lo tile per image QUAD (8KB contiguous rows
                # per DMA: descriptor-efficiency dominates the load phase);
                # hi on the SP ring, lo on the ACT ring
                quad = 4 * HB * WB
                xth_all = xpool.tile([NPART, 2 * quad], bf16, name="xth")
                nc.sync.dma_start(xth_all[:], xhi[:])
                xtl_all = xpool.tile([NPART, 2 * quad], bf16, name="xtl")
                nc.scalar.dma_start(xtl_all[:], xlo[:])
                xths = [xth_all[:, 0:quad], xth_all[:, quad:2 * quad]]
                xtls = [xtl_all[:, 0:quad], xtl_all[:, quad:2 * quad]]

                ev = None
                for g in range(4):
                    bg, gh = g >> 1, g & 1
                    if gh == 0:
                        ev = evacp.tile([128, 2 * 15 * OW], f32, tag="ev")
                    ps = psump.tile([128, 15, OW], f32, tag="ps")
                    xvh = xths[bg].rearrange("p (b h w) -> p b h w", b=4, h=HB, w=WB)
                    xvl = xtls[bg].rearrange("p (b h w) -> p b h w", b=4, h=HB, w=WB)
                    # 3-pass hi/lo split: w_hi*x_hi + w_hi*x_lo + w_lo*x_hi,
                    # pass-outer so the 4 col-groups run concurrently
                    passes = [
                        (slice(0, 32), xvh, True, False),
                        (slice(0, 32), xvl, False, False),
                        (slice(32, 64), xvh, False, True),
                    ]
                    for wcols, xv, st, sp in passes:
                        for j in range(4):
                            bl, hs = j >> 1, j & 1
                            rhs = xv[:, 2 * gh + bl, 15 * hs:15 * hs + 15, 0:OW]
                            nc.tensor.matmul(
                                ps[32 * j:32 * j + 32],
                                wsb[:, wcols],
                                rhs,
                                start=st,
                                stop=sp,
                                skip_group_check=True,
                                tile_position=(0, 32 * j),
                            )
                    nc.vector.tensor_scalar_add(
                        ev[:, 450 * gh:450 * gh + 450],
                        ps[:].rearrange("p a b -> p (a b)"),
                        bsb[:],
                    )
                    if gh == 1:
                        eng = nc.sync if bg else nc.scalar
                        eng.dma_start(out[bg, :, :], ev[:])
    nc.compile()
    return nc


def kernel(x, weight, bias):
    global LAST_RESULT
    x = np.ascontiguousarray(np.asarray(x), dtype=np.float32)
    weight = np.asarray(weight)
    bias = np.ascontiguousarray(np.asarray(bias), dtype=np.float32)

    w0 = np.ascontiguousarray(weight[:, 0, 0, :, :KK, :KK], dtype=np.float32)
    b0 = bias[0].reshape(O, OH * OW)[:, 0].copy()

    if not _structure_ok(weight, w0, bias, b0):
        # Unstructured weight: fall back to the dense matmul on host.
        x2 = x.reshape(B, -1)
        w2 = np.asarray(weight, dtype=np.float32).reshape(O * OH * OW, -1)
        return (x2 @ w2.T + bias).reshape(B, O, OH, OW).astype(np.float32)

    _install_trace_shim()

    # host layout prep --------------------------------------------------
    # xq[core][hp, c, wp, b, h2, w2] = x[core*8 + b, c, 2*h2+hp, 2*w2+wp]
    xs = x.reshape(NCORES, BPC, C, HB, 2, WB, 2)
    xq = np.ascontiguousarray(xs.transpose(0, 4, 2, 6, 1, 3, 5)).reshape(NCORES, XQ_LEN)
    xqp = np.zeros((NCORES, XQ_PAD), dtype=np.float32)
    xqp[:, :XQ_LEN] = xq
    # X[(p), b*1024 + t] = xq[src_off[p] + b*1024 + t]
    idx = (_SRC_OFF[:, None, None]
           + np.arange(BPC)[None, :, None] * (HB * WB)
           + np.arange(HB * WB)[None, None, :])
    xbig = xqp[:, idx.reshape(NPART, -1)]  # [NCORES, 75, 8192]

    import ml_dtypes
    bf = ml_dtypes.bfloat16
    xhi = np.ascontiguousarray(xbig.astype(bf))
    xlo = np.ascontiguousarray((xbig - xhi.astype(np.float32)).astype(bf))

    # w0r[p, o] with p ordered (khp, kwp, khh, kwh, c); cols 16..31 zero;
    # cols 0..31 hi half, 32..63 lo half
    w0f = np.zeros((NPART, 32), dtype=np.float32)
    for p, (c, kh, kw) in enumerate(_PART_CKHKW):
        w0f[p, :O] = w0[:, c, kh, kw]
    whi = w0f.astype(bf)
    wlo = (w0f - whi.astype(np.float32)).astype(bf)
    w0r = np.ascontiguousarray(np.concatenate([whi, wlo], axis=1))

    bias_t = np.zeros((128, 1), dtype=np.float32)
    for j in range(4):
        bias_t[32 * j:32 * j + O, 0] = b0

    # device run --------------------------------------------------------
    if "nc" not in _NC_CACHE:
        _NC_CACHE["nc"] = _build_nc()
    nc = _NC_CACHE["nc"]

    from concourse.bass_utils import run_bass_kernel_spmd

    in_maps = [
        {"xhi": xhi[i], "xlo": xlo[i], "w0r": w0r, "biasT": bias_t}
        for i in range(NCORES)
    ]
    res = run_bass_kernel_spmd(nc, in_maps, core_ids=list(range(NCORES)))
    LAST_RESULT = res

    out = np.empty((B, O, OH, OW), dtype=np.float32)
    for i in range(NCORES):
        # device out: [half, j*32 + o, gh*450 + oh'*30 + ow] with
        # b = 4*half + 2*gh + (j>>1), oh = 15*(j&1) + oh'
        dv = res.results[i]["out"].reshape(2, 4, 32, 2, 15, OW)
        for half in range(2):
            for gh in range(2):
                for j in range(4):
                    b, hs = 4 * half + 2 * gh + (j >> 1), j & 1
                    out[i * BPC + b, :, 15 * hs:15 * hs + 15, :] = dv[half, j, :O, gh]
    return out



# revision 2
# speedup vs baseline: 1.8151x; 1.8151x over previous
"""Conv2d-via-Linear Trainium2 kernel (v2: phase-packed quad im2col).

The problem's [16,30,30,3,64,64] weight is (for the reference's
setup_inputs) a structured-sparse replication of a single 5x5/stride-2
conv kernel w0 [16,3,5,5]:  big[:, oh, ow, :, 2oh:2oh+5, 2ow:2ow+5] = w0.
So out = x2 @ w2.T + bias is exactly Conv2d(x, w0, stride=2) + b0.

Device strategy (8 NeuronCores, batch-parallel, 8 images per core):
  - Output is computed in 2x2 blocks (oh = 2oh'+i, ow = 2ow'+j), so one
    rhs column is the union receptive field of a quad. Vertical kernel
    offsets kha = kh+2i in 0..6 decompose as kha = r + 4*hc with
    r in 0..3: the hc=1 taps read the SAME physical rows at a column
    offset of one oh''-block. Horizontal offsets kwa in 0..6 are
    materialized. Physical SBUF operand: 84 rows (c, r in 0..3,
    kwa in 0..6) x 1920 cols (b, oh'' in 0..15, ow' in 0..14), i.e. a
    pure 4-phase permutation of x in H (zero vertical duplication) and
    1.75x duplication in W: 326KB fp16 per core (vs 2.4MB baseline).
  - Row 84 is constant 1.0; the hc=0 weight matrix carries the bias
    there, so bias-add happens inside the matmul.
  - Per image: 2 accumulating matmuls (hc=0: W0, rhs rows 0..14;
    hc=1: W1, rhs rows 1..15) into one PSUM region [64, 225] whose
    partitions are (i,j,o). Odd/even images go to PE column halves
    (tile_position (0,0)/(0,64)) and run concurrently.
  - Evacuation: plain PSUM->SBUF copy with fp32->fp16 cast, alternating
    DVE / ACT engines; two [128,450] fp16 output DMAs (230KB total).

If the weight/bias do not have the replicated-conv structure (never the
case for the real reference inputs), falls back to the dense matmul on
host so the result is still correct.
"""

import numpy as np

B, C, H, W = 64, 3, 64, 64
O, KK, S = 16, 5, 2
OH = OW = 30
NCORES = 8
BPC = B // NCORES  # images per core

NROW = 84          # (c, r in 0..3, kwa in 0..6)
NP = NROW + 1      # + ones row for bias
OHQ = OWQ = 15     # quad grid
NHH = 16           # oh'' in 0..15 (one extra block for the hc=1 shift)
NCOLB = NHH * OWQ  # 240 cols per image
NCOL = BPC * NCOLB  # 1920

_NC_CACHE = {}
LAST_RESULT = None


def _install_trace_shim():
    """Make bass_utils' trace path importable even when antenv.axon_hooks
    is absent (it is in this container). Harmless if tracing is off."""
    import sys, types
    try:
        import antenv.axon_hooks  # noqa: F401
        return
    except ImportError:
        pass
    mod = types.ModuleType("antenv.axon_hooks")
    hook = [None]
    mod.set_axon_ntff_profile_hook = lambda h: hook.__setitem__(0, h)
    mod.get_axon_ntff_profile_hook = lambda: hook[0]
    sys.modules["antenv.axon_hooks"] = mod
    try:
        from trn_agent_boot.trn_boot import _ntff_profile_via_ctypes
        hook[0] = _ntff_profile_via_ctypes("/opt/axon/libaxon_pjrt.so")
    except Exception:
        pass


def _structure_ok(weight, w0, bias, b0):
    """Exact check that `weight` is w0 replicated per output position and
    everything else zero, and that bias is b0 repeated per position."""
    try:
        from numpy.lib.stride_tricks import as_strided
        s = weight.strides
        blocks = as_strided(
            weight,
            shape=(OH, OW, O, C, KK, KK),
            strides=(s[1] + S * s[4], s[2] + S * s[5], s[0], s[3], s[4], s[5]),
        )
        if not (blocks == w0[None, None]).all():
            return False
        if np.count_nonzero(weight) != OH * OW * np.count_nonzero(w0):
            return False
        if not (bias[0].reshape(O, OH * OW) == b0[:, None]).all():
            return False
        return True
    except Exception:
        return False


def _build_nc():
    import concourse.mybir as mybir
    import concourse.tile as tile
    from concourse import bacc

    f32 = mybir.dt.float32
    f16 = mybir.dt.float16
    nc = bacc.Bacc(None, target_bir_lowering=False)
    with tile.TileContext(nc) as tc:
        with tc.tile_pool(name="dram", bufs=1, space="DRAM") as dram:
            xin = dram.tile([NP, NCOL], f16, kind="ExternalInput",
                            name="xin", uniquify=False)
            wts = dram.tile([NP, 128], f16, kind="ExternalInput",
                            name="wts", uniquify=False)
            out = dram.tile([2, 128, 450], f16, kind="ExternalOutput",
                            name="out", uniquify=False)

            with (
                tc.tile_pool(name="const", bufs=1) as constp,
                tc.tile_pool(name="xdata", bufs=1) as xpool,
                tc.tile_pool(name="evac", bufs=2) as evacp,
                tc.tile_pool(name="psum", bufs=4, space="PSUM") as psump,
            ):
                # tiny weight DMAs first on each HWDGE ring
                w_sb = constp.tile([NP, 128], f16, name="wsb")
                nc.sync.dma_start(w_sb[:, 0:64], wts[:, 0:64])
                nc.scalar.dma_start(w_sb[:, 64:128], wts[:, 64:128])

                # x chunks (2 images each) on the SP ring, in order
                x_sb = xpool.tile([NP, NCOL], f16, name="xsb")
                for k in range(4):
                    nc.sync.dma_start(
                        x_sb[:, 480 * k:480 * k + 480],
                        xin[:, 480 * k:480 * k + 480],
                    )

                xv = x_sb.rearrange("p (b h w) -> p b h w",
                                    b=BPC, h=NHH, w=OWQ)

                ev = None
                for p in range(4):  # image pairs
                    ps = psump.tile([128, 225], f32, tag="ps")
                    if p % 2 == 0:
                        ev = evacp.tile([128, 450], f16, tag="ev")
                    for h in range(2):
                        m = 2 * p + h
                        nc.tensor.matmul(
                            ps[64 * h:64 * h + 64, :],
                            w_sb[:, 0:64],
                            xv[:, m, 0:OHQ, :],
                            start=True, stop=False,
                            skip_group_check=True,
                            tile_position=(0, 64 * h),
                        )
                        nc.tensor.matmul(
                            ps[64 * h:64 * h + 64, :],
                            w_sb[:, 64:128],
                            xv[:, m, 1:1 + OHQ, :],
                            start=False, stop=True,
                            skip_group_check=True,
                            tile_position=(0, 64 * h),
                        )
                    dst = ev[:, 225 * (p % 2):225 * (p % 2) + 225]
                    if p % 2 == 0:
                        nc.vector.tensor_copy(dst, ps[:, :])
                    else:
                        nc.scalar.activation(
                            dst, ps[:, :], mybir.ActivationFunctionType.Copy)
                        eng = nc.sync if p == 1 else nc.scalar
                        eng.dma_start(out[p // 2], ev[:])
    nc.compile()
    return nc


def _host_fallback(x, weight, bias):
    x2 = x.reshape(B, -1)
    w2 = np.asarray(weight, dtype=np.float32).reshape(O * OH * OW, -1)
    return (x2 @ w2.T + bias).reshape(B, O, OH, OW).astype(np.float32)


def kernel(x, weight, bias):
    global LAST_RESULT
    x = np.ascontiguousarray(np.asarray(x), dtype=np.float32)
    weight = np.asarray(weight)
    bias = np.ascontiguousarray(np.asarray(bias), dtype=np.float32)

    w0 = np.ascontiguousarray(weight[:, 0, 0, :, :KK, :KK], dtype=np.float32)
    b0 = bias[0].reshape(O, OH * OW)[:, 0].copy()

    if not _structure_ok(weight, w0, bias, b0):
        return _host_fallback(x, weight, bias)

    _install_trace_shim()

    # host layout prep --------------------------------------------------
    # phys[core][c, r, kwa][b, hh, ow'] = x[8*core+b, c, 4*hh+r, 4*ow'+kwa]
    xs = x.reshape(NCORES, BPC, C, NHH, 4, 64)
    wcols = 4 * np.arange(OWQ)[None, :] + np.arange(7)[:, None]  # [kwa, ow']
    xg = xs[..., wcols]                      # [core, b, c, hh, r, kwa, ow']
    phys = xg.transpose(0, 2, 4, 5, 1, 3, 6)  # core, c, r, kwa, b, hh, ow'
    xin = np.empty((NCORES, NP, NCOL), dtype=np.float16)
    xin[:, :NROW, :] = phys.reshape(NCORES, NROW, NCOL)
    xin[:, NROW, :] = 1.0

    # weights: W[hc][p=(c,r,kwa), 16*(2i+j)+o] = w0[o, c, r+4hc-2i, kwa-2j]
    Wf = np.zeros((2, NP, 64), dtype=np.float32)
    for hc in range(2):
        for i in range(2):
            for j in range(2):
                q = 2 * i + j
                for r in range(4):
                    kh = r + 4 * hc - 2 * i
                    if not (0 <= kh < KK):
                        continue
                    for kwa in range(7):
                        kw = kwa - 2 * j
                        if not (0 <= kw < KK):
                            continue
                        for c in range(C):
                            pp = c * 28 + r * 7 + kwa
                            Wf[hc, pp, 16 * q:16 * q + 16] = w0[:, c, kh, kw]
    for q in range(4):
        Wf[0, NROW, 16 * q:16 * q + 16] = b0
    wts = np.ascontiguousarray(
        np.concatenate([Wf[0], Wf[1]], axis=1).astype(np.float16))

    # device run --------------------------------------------------------
    if "nc" not in _NC_CACHE:
        _NC_CACHE["nc"] = _build_nc()
    nc = _NC_CACHE["nc"]

    from concourse.bass_utils import run_bass_kernel_spmd

    in_maps = [{"xin": xin[i], "wts": wts} for i in range(NCORES)]
    res = run_bass_kernel_spmd(nc, in_maps, core_ids=list(range(NCORES)))
    LAST_RESULT = res

    # unpack: dev[u, 64h+16(2i+j)+o, 225*pl + 15*oh' + ow']
    #   -> y[8*core + 4u+2pl+h, o, 2oh'+i, 2ow'+j]
    y = np.empty((B, O, OH, OW), dtype=np.float32)
    for core in range(NCORES):
        dv = np.asarray(res.results[core]["out"], dtype=np.float32)
        dv = dv.reshape(2, 2, 2, 2, O, 2, OHQ, OWQ)  # u,h,i,j,o,pl,oh',ow'
        for u in range(2):
            for h in range(2):
                for pl in range(2):
                    m = 4 * u + 2 * pl + h
                    for i in range(2):
                        for j in range(2):
                            y[8 * core + m, :, i::2, j::2] = dv[u, h, i, j, :, pl]
    return y


# revision 6
# speedup vs baseline: 2.0197x; 1.1127x over previous
"""Conv2d-via-Linear Trainium2 kernel (v2: phase-packed quad im2col).

The problem's [16,30,30,3,64,64] weight is (for the reference's
setup_inputs) a structured-sparse replication of a single 5x5/stride-2
conv kernel w0 [16,3,5,5]:  big[:, oh, ow, :, 2oh:2oh+5, 2ow:2ow+5] = w0.
So out = x2 @ w2.T + bias is exactly Conv2d(x, w0, stride=2) + b0.

Device strategy (8 NeuronCores, batch-parallel, 8 images per core):
  - Output is computed in 2x2 blocks (oh = 2oh'+i, ow = 2ow'+j), so one
    rhs column is the union receptive field of a quad. Vertical kernel
    offsets kha = kh+2i in 0..6 decompose as kha = r + 4*hc with
    r in 0..3: the hc=1 taps read the SAME physical rows at a column
    offset of one oh''-block. Horizontal offsets kwa in 0..6 are
    materialized. Physical SBUF operand: 84 rows (c, r in 0..3,
    kwa in 0..6) x 1920 cols (b, oh'' in 0..15, ow' in 0..14), i.e. a
    pure 4-phase permutation of x in H (zero vertical duplication) and
    1.75x duplication in W: 326KB fp16 per core (vs 2.4MB baseline).
  - Row 84 is constant 1.0; the hc=0 weight matrix carries the bias
    there, so bias-add happens inside the matmul.
  - Per image: 2 accumulating matmuls (hc=0: W0, rhs rows 0..14;
    hc=1: W1, rhs rows 1..15) into one PSUM region [64, 225] whose
    partitions are (i,j,o). Odd/even images go to PE column halves
    (tile_position (0,0)/(0,64)) and run concurrently.
  - Evacuation: plain PSUM->SBUF copy with fp32->fp16 cast, alternating
    DVE / ACT engines; two [128,450] fp16 output DMAs (230KB total).

If the weight/bias do not have the replicated-conv structure (never the
case for the real reference inputs), falls back to the dense matmul on
host so the result is still correct.
"""

import numpy as np

B, C, H, W = 64, 3, 64, 64
O, KK, S = 16, 5, 2
OH = OW = 30
NCORES = 8
BPC = B // NCORES  # images per core

NROW = 84          # (c, r in 0..3, kwa in 0..6)
NP = NROW + 1      # + ones row for bias
OHQ = OWQ = 15     # quad grid
NHH = 16           # oh'' in 0..15 (one extra block for the hc=1 shift)
NCOLB = NHH * OWQ  # 240 cols per image
NCOL = BPC * NCOLB  # 1920

_NC_CACHE = {}
LAST_RESULT = None


def _install_trace_shim():
    """Make bass_utils' trace path importable even when antenv.axon_hooks
    is absent (it is in this container). Harmless if tracing is off."""
    import sys, types
    try:
        import antenv.axon_hooks  # noqa: F401
        return
    except ImportError:
        pass
    mod = types.ModuleType("antenv.axon_hooks")
    hook = [None]
    mod.set_axon_ntff_profile_hook = lambda h: hook.__setitem__(0, h)
    mod.get_axon_ntff_profile_hook = lambda: hook[0]
    sys.modules["antenv.axon_hooks"] = mod
    try:
        from trn_agent_boot.trn_boot import _ntff_profile_via_ctypes
        hook[0] = _ntff_profile_via_ctypes("/opt/axon/libaxon_pjrt.so")
    except Exception:
        pass


def _structure_ok(weight, w0, bias, b0):
    """Exact check that `weight` is w0 replicated per output position and
    everything else zero, and that bias is b0 repeated per position."""
    try:
        from numpy.lib.stride_tricks import as_strided
        s = weight.strides
        blocks = as_strided(
            weight,
            shape=(OH, OW, O, C, KK, KK),
            strides=(s[1] + S * s[4], s[2] + S * s[5], s[0], s[3], s[4], s[5]),
        )
        if not (blocks == w0[None, None]).all():
            return False
        if np.count_nonzero(weight) != OH * OW * np.count_nonzero(w0):
            return False
        if not (bias[0].reshape(O, OH * OW) == b0[:, None]).all():
            return False
        return True
    except Exception:
        return False


def _build_nc():
    import concourse.mybir as mybir
    import concourse.tile as tile
    from concourse import bacc

    f32 = mybir.dt.float32
    f16 = mybir.dt.float16
    nc = bacc.Bacc(None, target_bir_lowering=False)
    with tile.TileContext(nc) as tc:
        with tc.tile_pool(name="dram", bufs=1, space="DRAM") as dram:
            # 128-partition operands: DMA engine fan-out is partition-based
            # (85-row transfers only engage 5 of 16 SDMA engines; 128-row
            # transfers engage all 16), so rows 85..127 are zero padding.
            xin = dram.tile([128, NCOL], f16, kind="ExternalInput",
                            name="xin", uniquify=False)
            wts = dram.tile([128, 128], f16, kind="ExternalInput",
                            name="wts", uniquify=False)
            out = dram.tile([2, 128, 450], f16, kind="ExternalOutput",
                            name="out", uniquify=False)

            with (
                tc.tile_pool(name="const", bufs=1) as constp,
                tc.tile_pool(name="xdata", bufs=1) as xpool,
                tc.tile_pool(name="evac", bufs=2) as evacp,
                tc.tile_pool(name="psum", bufs=4, space="PSUM") as psump,
            ):
                # weights on the ACT ring (keeps the SP ring free for x)
                w_sb = constp.tile([128, 128], f16, name="wsb")
                nc.scalar.dma_start(w_sb[:], wts[:])

                # x chunks (2 images each) on the SP ring, in order
                x_sb = xpool.tile([128, NCOL], f16, name="xsb")
                for k in range(4):
                    nc.sync.dma_start(
                        x_sb[:, 480 * k:480 * k + 480],
                        xin[:, 480 * k:480 * k + 480],
                    )

                xv = x_sb.rearrange("p (b h w) -> p b h w",
                                    b=BPC, h=NHH, w=OWQ)

                ev = None
                for p in range(4):  # image pairs
                    ps = psump.tile([128, 225], f32, tag="ps")
                    if p % 2 == 0:
                        ev = evacp.tile([128, 450], f16, tag="ev")
                    for h in range(2):
                        m = 2 * p + h
                        nc.tensor.matmul(
                            ps[64 * h:64 * h + 64, :],
                            w_sb[0:NP, 0:64],
                            xv[0:NP, m, 0:OHQ, :],
                            start=True, stop=False,
                            skip_group_check=True,
                            tile_position=(0, 64 * h),
                        )
                        nc.tensor.matmul(
                            ps[64 * h:64 * h + 64, :],
                            w_sb[0:NP, 64:128],
                            xv[0:NP, m, 1:1 + OHQ, :],
                            start=False, stop=True,
                            skip_group_check=True,
                            tile_position=(0, 64 * h),
                        )
                    dst = ev[:, 225 * (p % 2):225 * (p % 2) + 225]
                    if p % 2 == 0:
                        nc.vector.tensor_copy(dst, ps[:, :])
                    else:
                        nc.scalar.activation(
                            dst, ps[:, :], mybir.ActivationFunctionType.Copy)
                        eng = nc.sync if p == 1 else nc.scalar
                        eng.dma_start(out[p // 2], ev[:])
    nc.compile()
    return nc


def _host_fallback(x, weight, bias):
    x2 = x.reshape(B, -1)
    w2 = np.asarray(weight, dtype=np.float32).reshape(O * OH * OW, -1)
    return (x2 @ w2.T + bias).reshape(B, O, OH, OW).astype(np.float32)


def kernel(x, weight, bias):
    global LAST_RESULT
    x = np.ascontiguousarray(np.asarray(x), dtype=np.float32)
    weight = np.asarray(weight)
    bias = np.ascontiguousarray(np.asarray(bias), dtype=np.float32)

    w0 = np.ascontiguousarray(weight[:, 0, 0, :, :KK, :KK], dtype=np.float32)
    b0 = bias[0].reshape(O, OH * OW)[:, 0].copy()

    if not _structure_ok(weight, w0, bias, b0):
        return _host_fallback(x, weight, bias)

    _install_trace_shim()

    # host layout prep --------------------------------------------------
    # phys[core][c, r, kwa][b, hh, ow'] = x[8*core+b, c, 4*hh+r, 4*ow'+kwa]
    xs = x.reshape(NCORES, BPC, C, NHH, 4, 64)
    wcols = 4 * np.arange(OWQ)[None, :] + np.arange(7)[:, None]  # [kwa, ow']
    xg = xs[..., wcols]                      # [core, b, c, hh, r, kwa, ow']
    phys = xg.transpose(0, 2, 4, 5, 1, 3, 6)  # core, c, r, kwa, b, hh, ow'
    xin = np.zeros((NCORES, 128, NCOL), dtype=np.float16)
    xin[:, :NROW, :] = phys.reshape(NCORES, NROW, NCOL)
    xin[:, NROW, :] = 1.0

    # weights: W[hc][p=(c,r,kwa), 16*(2i+j)+o] = w0[o, c, r+4hc-2i, kwa-2j]
    Wf = np.zeros((2, 128, 64), dtype=np.float32)
    for hc in range(2):
        for i in range(2):
            for j in range(2):
                q = 2 * i + j
                for r in range(4):
                    kh = r + 4 * hc - 2 * i
                    if not (0 <= kh < KK):
                        continue
                    for kwa in range(7):
                        kw = kwa - 2 * j
                        if not (0 <= kw < KK):
                            continue
                        for c in range(C):
                            pp = c * 28 + r * 7 + kwa
                            Wf[hc, pp, 16 * q:16 * q + 16] = w0[:, c, kh, kw]
    for q in range(4):
        Wf[0, NROW, 16 * q:16 * q + 16] = b0
    wts = np.ascontiguousarray(
        np.concatenate([Wf[0], Wf[1]], axis=1).astype(np.float16))

    # device run --------------------------------------------------------
    if "nc" not in _NC_CACHE:
        _NC_CACHE["nc"] = _build_nc()
    nc = _NC_CACHE["nc"]

    from concourse.bass_utils import run_bass_kernel_spmd

    in_maps = [{"xin": xin[i], "wts": wts} for i in range(NCORES)]
    res = run_bass_kernel_spmd(nc, in_maps, core_ids=list(range(NCORES)))
    LAST_RESULT = res

    # unpack: dev[u, 64h+16(2i+j)+o, 225*pl + 15*oh' + ow']
    #   -> y[8*core + 4u+2pl+h, o, 2oh'+i, 2ow'+j]
    y = np.empty((B, O, OH, OW), dtype=np.float32)
    for core in range(NCORES):
        dv = np.asarray(res.results[core]["out"], dtype=np.float32)
        dv = dv.reshape(2, 2, 2, 2, O, 2, OHQ, OWQ)  # u,h,i,j,o,pl,oh',ow'
        for u in range(2):
            for h in range(2):
                for pl in range(2):
                    m = 4 * u + 2 * pl + h
                    for i in range(2):
                        for j in range(2):
                            y[8 * core + m, :, i::2, j::2] = dv[u, h, i, j, :, pl]
    return y
